# revision 6
# baseline (speedup 1.0000x reference)
"""Sliding-window causal attention (window=512) on 8 TRN2 NeuronCores.

Full inputs q,k,v: [4, 16, 2048, 128] fp32. B*H = 64 (batch, head) pairs are
sharded 8-per-core (head/batch parallel, no cross-core communication).

Per (pair, 128-query-block) on device:
  - <=5 QK^T matmuls (bf16) produce transposed scores S^T[key, q] in PSUM
    (key on partitions so the AV matmul needs no on-chip transpose).
  - one Exp over the whole score block (scores are O(1): q is pre-scaled by
    1/sqrt(d) on host, so no max-subtraction is needed).
  - triangular 0/1 mask multiplies on the first (window-edge) and diagonal
    (causal) key blocks.
  - <=5 accumulating AV matmuls: out[q, 0:128] = P^T.T @ v, out[q, 128] = sum
    of probs (denominator) via a ones-column appended to v on host.
  - normalization (divide by out[:, 128]) happens on host after gather.

Host-side prep/post (numpy) handles the [T,d] -> [d,T] transposes, bf16
casts, and the final division - none of which touch the device.
"""

import os

import ml_dtypes
import numpy as np

from concourse import bacc, bass, mybir, tile
from concourse.bass_utils import run_bass_kernel_spmd

B, H, T, D = 4, 16, 2048, 128
WINDOW = 512
SCALE = D ** -0.5
N_CORES = 8
PAIRS_PER_CORE = (B * H) // N_CORES  # 8
NQB = T // 128                       # 16 query blocks of 128 per pair
NKB = T // 128                       # 16 key blocks of 128 per pair
VSLOT = 129                          # v block width + ones column
BF16 = mybir.dt.bfloat16
F32 = mybir.dt.float32

_TRACE = bool(int(os.environ.get("KERNEL_TRACE", "0")))
LAST_RUN_INFO = {}


def _ensure_ntff_hook():
    """The agent image's ``antenv`` lacks ``axon_hooks``, so concourse's
    trace path can't find the NTFF profile hook. Synthesize the module and
    register the ctypes-based hook from trn_agent_boot."""
    import sys
    import types

    try:
        from antenv.axon_hooks import get_axon_ntff_profile_hook  # noqa: F401
        return True
    except ImportError:
        pass
    try:
        import antenv
        from trn_agent_boot.trn_boot import _ntff_profile_via_ctypes

        hook = _ntff_profile_via_ctypes("/opt/axon/libaxon_pjrt.so")
        mod = types.ModuleType("antenv.axon_hooks")
        _state = {"hook": hook}
        mod.set_axon_ntff_profile_hook = lambda h: _state.__setitem__("hook", h)
        mod.get_axon_ntff_profile_hook = lambda: _state["hook"]
        sys.modules["antenv.axon_hooks"] = mod
        antenv.axon_hooks = mod
        return hook is not None
    except Exception:
        return False


def _build_bass():
    # bacc.Bacc (not bass.Bass): its finalize() runs
    # generate_event_semaphores(), which splits multi-sem waits to satisfy
    # the TRN2 one-wait-per-instruction constraint walrus enforces.
    nc = bacc.Bacc()
    qT_ext = nc.declare_dram_parameter(
        "qT", [PAIRS_PER_CORE, 128, T], BF16, isOutput=False)
    kT_ext = nc.declare_dram_parameter(
        "kT", [PAIRS_PER_CORE, 128, T], BF16, isOutput=False)
    v_ext = nc.declare_dram_parameter(
        "vext", [PAIRS_PER_CORE, 128, NKB * VSLOT], BF16, isOutput=False)
    m_ext = nc.declare_dram_parameter("masks", [128, 256], BF16, isOutput=False)
    out_ext = nc.declare_dram_parameter(
        "out", [PAIRS_PER_CORE, 128, NQB * VSLOT], BF16, isOutput=True)

    with tile.TileContext(nc) as tc:
        with (
            tc.tile_pool(name="qk_in", bufs=2) as qk_pool,
            tc.tile_pool(name="v_in", bufs=2) as v_pool,
            tc.tile_pool(name="mask", bufs=1) as mask_pool,
            tc.tile_pool(name="probs", bufs=3) as probs_pool,
            tc.tile_pool(name="stage", bufs=2) as stage_pool,
            tc.tile_pool(name="scores", bufs=2, space="PSUM") as scores_pool,
            tc.tile_pool(name="outp", bufs=2, space="PSUM") as outp_pool,
        ):
            mask_t = mask_pool.tile([128, 256], BF16)
            nc.sync.dma_start(mask_t[:], m_ext[:])
            m2 = mask_t[:, 0:128]    # window edge block: valid iff r < s
            m1 = mask_t[:, 128:256]  # diagonal block: valid iff r >= s

            for p in range(PAIRS_PER_CORE):
                qt = qk_pool.tile([128, T], BF16, tag="qt")
                nc.sync.dma_start(qt[:], qT_ext[p])
                kt = qk_pool.tile([128, T], BF16, tag="kt")
                nc.sync.dma_start(kt[:], kT_ext[p])
                vt = v_pool.tile([128, NKB * VSLOT], BF16, tag="vt")
                nc.sync.dma_start(vt[:], v_ext[p])
                stage = stage_pool.tile([128, NQB * VSLOT], BF16, tag="stage")

                for qi in range(NQB):
                    kb0 = max(0, qi - 4)
                    nkb = qi - kb0 + 1
                    scores = scores_pool.tile([128, nkb * 128], F32, tag="scores")
                    for j in range(nkb):
                        nc.tensor.matmul(
                            scores[:, j * 128:(j + 1) * 128],
                            lhsT=kt[:, (kb0 + j) * 128:(kb0 + j + 1) * 128],
                            rhs=qt[:, qi * 128:(qi + 1) * 128],
                            start=True, stop=True,
                        )
                    probs = probs_pool.tile([128, nkb * 128], BF16, tag="probs")
                    nc.scalar.activation(
                        probs[:], scores[:], mybir.ActivationFunctionType.Exp)
                    if qi >= 4:
                        nc.vector.tensor_mul(probs[:, 0:128], probs[:, 0:128], m2)
                    nc.vector.tensor_mul(
                        probs[:, (nkb - 1) * 128:nkb * 128],
                        probs[:, (nkb - 1) * 128:nkb * 128], m1)
                    outp = outp_pool.tile([128, VSLOT], F32, tag="outp")
                    for j in range(nkb):
                        nc.tensor.matmul(
                            outp[:],
                            lhsT=probs[:, j * 128:(j + 1) * 128],
                            rhs=vt[:, (kb0 + j) * VSLOT:(kb0 + j) * VSLOT + VSLOT],
                            start=(j == 0), stop=(j == nkb - 1),
                        )
                    nc.vector.tensor_copy(
                        stage[:, qi * VSLOT:(qi + 1) * VSLOT], outp[:])

                nc.sync.dma_start(out_ext[p], stage[:])

    # Run bacc's lowering (register allocation + sem-wait legalization);
    # run_bass_via_pjrt serializes without finalizing.
    nc.finalize()
    return nc


_NC_CACHE = None


def _get_nc():
    global _NC_CACHE
    if _NC_CACHE is None:
        _NC_CACHE = _build_bass()
    return _NC_CACHE


def kernel(q, k, v):
    q = np.asarray(q, dtype=np.float32)
    k = np.asarray(k, dtype=np.float32)
    v = np.asarray(v, dtype=np.float32)
    bf16 = ml_dtypes.bfloat16

    npairs = B * H
    # [pairs, d, T] transposed layouts for the QK^T matmul; q pre-scaled.
    qT = np.ascontiguousarray(
        (q.reshape(npairs, T, D) * SCALE).transpose(0, 2, 1)).astype(bf16)
    kT = np.ascontiguousarray(
        k.reshape(npairs, T, D).transpose(0, 2, 1)).astype(bf16)
    # v blocks in natural layout + ones column: vext[p, s, kb*129 + c]
    vext = np.ones((npairs, 128, NKB, VSLOT), dtype=np.float32)
    vext[:, :, :, :D] = v.reshape(npairs, NKB, 128, D).transpose(0, 2, 1, 3)
    vext = vext.reshape(npairs, 128, NKB * VSLOT).astype(bf16)

    s_idx = np.arange(128)[:, None]
    r_idx = np.arange(128)[None, :]
    masks = np.zeros((128, 256), dtype=np.float32)
    masks[:, 0:128] = (r_idx < s_idx)    # M2: window edge block
    masks[:, 128:256] = (r_idx >= s_idx)  # M1: causal diagonal block
    masks = masks.astype(bf16)

    in_maps = []
    for c in range(N_CORES):
        lo, hi = c * PAIRS_PER_CORE, (c + 1) * PAIRS_PER_CORE
        in_maps.append({
            "qT": qT[lo:hi], "kT": kT[lo:hi], "vext": vext[lo:hi],
            "masks": masks,
        })

    nc = _get_nc()
    trace = _TRACE and _ensure_ntff_hook()
    res = run_bass_kernel_spmd(
        nc, in_maps, core_ids=list(range(N_CORES)), trace=trace)
    LAST_RUN_INFO["exec_time_ns"] = res.exec_time_ns
    LAST_RUN_INFO["mean_exec_time_ns"] = res.mean_exec_time_ns
    LAST_RUN_INFO["profile_json"] = res.profile_json

    # Gather + normalize + undo layouts on host.
    raw = np.concatenate(
        [np.asarray(res.results[c]["out"]) for c in range(N_CORES)], axis=0
    ).astype(np.float32)                              # [pairs, 128, NQB*129]
    raw = raw.reshape(npairs, 128, NQB, VSLOT)
    num = raw[:, :, :, :D]                            # [pairs, r, qi, d]
    den = raw[:, :, :, D:D + 1]
    out = (num / den).transpose(0, 2, 1, 3)           # [pairs, qi, r, d]
    return np.ascontiguousarray(
        out.reshape(B, H, T, D).astype(np.float32))


# revision 8
# speedup vs baseline: 1.0750x; 1.0750x over previous
"""Sliding-window causal attention (window=512) on 8 TRN2 NeuronCores.

Full inputs q,k,v: [4, 16, 2048, 128] fp32. B*H = 64 (batch, head) pairs are
sharded 8-per-core (head/batch parallel, no cross-core communication).

Per (pair, 128-query-block) on device:
  - <=5 QK^T matmuls (bf16) produce transposed scores S^T[key, q] in PSUM
    (key on partitions so the AV matmul needs no on-chip transpose).
  - one Exp over the whole score block (scores are O(1): q is pre-scaled by
    1/sqrt(d) on host, so no max-subtraction is needed).
  - triangular 0/1 mask multiplies on the first (window-edge) and diagonal
    (causal) key blocks.
  - <=5 accumulating AV matmuls: out[q, 0:128] = P^T.T @ v, out[q, 128] = sum
    of probs (denominator) via a ones-column appended to v on host.
  - normalization (divide by out[:, 128]) happens on host after gather.

Host-side prep/post (numpy) handles the [T,d] -> [d,T] transposes, bf16
casts, and the final division - none of which touch the device.
"""

import os

import ml_dtypes
import numpy as np

from concourse import bacc, bass, mybir, tile
from concourse.bass_utils import run_bass_kernel_spmd

B, H, T, D = 4, 16, 2048, 128
WINDOW = 512
SCALE = D ** -0.5
N_CORES = 8
PAIRS_PER_CORE = (B * H) // N_CORES  # 8
NQB = T // 128                       # 16 query blocks of 128 per pair
NKB = T // 128                       # 16 key blocks of 128 per pair
VSLOT = 129                          # v block width + ones column
BF16 = mybir.dt.bfloat16
F32 = mybir.dt.float32

_TRACE = bool(int(os.environ.get("KERNEL_TRACE", "0")))
LAST_RUN_INFO = {}


def _ensure_ntff_hook():
    """The agent image's ``antenv`` lacks ``axon_hooks``, so concourse's
    trace path can't find the NTFF profile hook. Synthesize the module and
    register the ctypes-based hook from trn_agent_boot."""
    import sys
    import types

    try:
        from antenv.axon_hooks import get_axon_ntff_profile_hook  # noqa: F401
        return True
    except ImportError:
        pass
    try:
        import antenv
        from trn_agent_boot.trn_boot import _ntff_profile_via_ctypes

        hook = _ntff_profile_via_ctypes("/opt/axon/libaxon_pjrt.so")
        mod = types.ModuleType("antenv.axon_hooks")
        _state = {"hook": hook}
        mod.set_axon_ntff_profile_hook = lambda h: _state.__setitem__("hook", h)
        mod.get_axon_ntff_profile_hook = lambda: _state["hook"]
        sys.modules["antenv.axon_hooks"] = mod
        antenv.axon_hooks = mod
        return hook is not None
    except Exception:
        return False


def _build_bass():
    # bacc.Bacc (not bass.Bass): its finalize() runs
    # generate_event_semaphores(), which splits multi-sem waits to satisfy
    # the TRN2 one-wait-per-instruction constraint walrus enforces.
    nc = bacc.Bacc()
    qT_ext = nc.declare_dram_parameter(
        "qT", [PAIRS_PER_CORE, 128, T], BF16, isOutput=False)
    kT_ext = nc.declare_dram_parameter(
        "kT", [PAIRS_PER_CORE, 128, T], BF16, isOutput=False)
    v_ext = nc.declare_dram_parameter(
        "vext", [PAIRS_PER_CORE, 128, NKB * VSLOT], BF16, isOutput=False)
    m_ext = nc.declare_dram_parameter("masks", [128, 256], BF16, isOutput=False)
    out_ext = nc.declare_dram_parameter(
        "out", [PAIRS_PER_CORE, 128, NQB * VSLOT], BF16, isOutput=True)

    with tile.TileContext(nc) as tc:
        with (
            tc.tile_pool(name="qk_in", bufs=2) as qk_pool,
            tc.tile_pool(name="v_in", bufs=2) as v_pool,
            tc.tile_pool(name="mask", bufs=1) as mask_pool,
            tc.tile_pool(name="probs", bufs=3) as probs_pool,
            tc.tile_pool(name="stage", bufs=2) as stage_pool,
            tc.tile_pool(name="scores", bufs=2, space="PSUM") as scores_pool,
            tc.tile_pool(name="outp", bufs=2, space="PSUM") as outp_pool,
        ):
            mask_t = mask_pool.tile([128, 256], BF16)
            nc.sync.dma_start(mask_t[:], m_ext[:])
            m2 = mask_t[:, 0:128]    # window edge block: valid iff r < s
            m1 = mask_t[:, 128:256]  # diagonal block: valid iff r >= s

            for p in range(PAIRS_PER_CORE):
                qt = qk_pool.tile([128, T], BF16, tag="qt")
                nc.sync.dma_start(qt[:], qT_ext[p])
                kt = qk_pool.tile([128, T], BF16, tag="kt")
                nc.sync.dma_start(kt[:], kT_ext[p])
                vt = v_pool.tile([128, NKB * VSLOT], BF16, tag="vt")
                nc.sync.dma_start(vt[:], v_ext[p])
                stage = stage_pool.tile([128, NQB * VSLOT], BF16, tag="stage")

                # Two query blocks per iteration: one exp + one PSUM out tile
                # + one stage copy per super-block halves ACT/DVE op-count
                # overhead and semaphore traffic.
                for qs in range(NQB // 2):
                    qiA, qiB = 2 * qs, 2 * qs + 1
                    kb0A = max(0, qiA - 4)
                    kb0B = max(0, qiB - 4)
                    nkbA = qiA - kb0A + 1
                    nkbB = qiB - kb0B + 1
                    wA = nkbA * 128
                    wtot = (nkbA + nkbB) * 128
                    scores = scores_pool.tile([128, wtot], F32, tag="scores")
                    for j in range(nkbA):
                        nc.tensor.matmul(
                            scores[:, j * 128:(j + 1) * 128],
                            lhsT=kt[:, (kb0A + j) * 128:(kb0A + j + 1) * 128],
                            rhs=qt[:, qiA * 128:(qiA + 1) * 128],
                            start=True, stop=True,
                        )
                    for j in range(nkbB):
                        nc.tensor.matmul(
                            scores[:, wA + j * 128:wA + (j + 1) * 128],
                            lhsT=kt[:, (kb0B + j) * 128:(kb0B + j + 1) * 128],
                            rhs=qt[:, qiB * 128:(qiB + 1) * 128],
                            start=True, stop=True,
                        )
                    probs = probs_pool.tile([128, wtot], BF16, tag="probs")
                    nc.scalar.activation(
                        probs[:], scores[:], mybir.ActivationFunctionType.Exp)

                    # Masks as one strided op over both blocks where possible.
                    diagA = (nkbA - 1) * 128
                    diagB = wA + (nkbB - 1) * 128
                    stride = diagB - diagA
                    def two_block_view(ap_full, col0, step):
                        base = ap_full[:, col0:col0 + 128]
                        return bass.AP(
                            base.tensor, base.offset,
                            [base.ap[0], [step, 2], [1, 128]])

                    m1b = bass.AP(
                        m1.tensor, m1.offset, [m1.ap[0], [0, 2]] + list(m1.ap[1:]))
                    diag2 = two_block_view(probs, diagA, stride)
                    nc.vector.tensor_mul(diag2, diag2, m1b)
                    if qiA >= 4:
                        # window-edge blocks sit at cols 0 (A) and wA (B)
                        m2b = bass.AP(
                            m2.tensor, m2.offset, [m2.ap[0], [0, 2]] + list(m2.ap[1:]))
                        edge2 = two_block_view(probs, 0, wA)
                        nc.vector.tensor_mul(edge2, edge2, m2b)

                    outp = outp_pool.tile([128, 2 * VSLOT], F32, tag="outp")
                    for j in range(nkbA):
                        nc.tensor.matmul(
                            outp[:, 0:VSLOT],
                            lhsT=probs[:, j * 128:(j + 1) * 128],
                            rhs=vt[:, (kb0A + j) * VSLOT:(kb0A + j + 1) * VSLOT],
                            start=(j == 0), stop=(j == nkbA - 1),
                        )
                    for j in range(nkbB):
                        nc.tensor.matmul(
                            outp[:, VSLOT:2 * VSLOT],
                            lhsT=probs[:, wA + j * 128:wA + (j + 1) * 128],
                            rhs=vt[:, (kb0B + j) * VSLOT:(kb0B + j + 1) * VSLOT],
                            start=(j == 0), stop=(j == nkbB - 1),
                        )
                    nc.vector.tensor_copy(
                        stage[:, qiA * VSLOT:(qiA + 2) * VSLOT], outp[:])

                nc.sync.dma_start(out_ext[p], stage[:])

    # Run bacc's lowering (register allocation + sem-wait legalization);
    # run_bass_via_pjrt serializes without finalizing.
    nc.finalize()
    return nc


_NC_CACHE = None


def _get_nc():
    global _NC_CACHE
    if _NC_CACHE is None:
        _NC_CACHE = _build_bass()
    return _NC_CACHE


def kernel(q, k, v):
    q = np.asarray(q, dtype=np.float32)
    k = np.asarray(k, dtype=np.float32)
    v = np.asarray(v, dtype=np.float32)
    bf16 = ml_dtypes.bfloat16

    npairs = B * H
    # [pairs, d, T] transposed layouts for the QK^T matmul; q pre-scaled.
    qT = np.ascontiguousarray(
        (q.reshape(npairs, T, D) * SCALE).transpose(0, 2, 1)).astype(bf16)
    kT = np.ascontiguousarray(
        k.reshape(npairs, T, D).transpose(0, 2, 1)).astype(bf16)
    # v blocks in natural layout + ones column: vext[p, s, kb*129 + c]
    vext = np.ones((npairs, 128, NKB, VSLOT), dtype=np.float32)
    vext[:, :, :, :D] = v.reshape(npairs, NKB, 128, D).transpose(0, 2, 1, 3)
    vext = vext.reshape(npairs, 128, NKB * VSLOT).astype(bf16)

    s_idx = np.arange(128)[:, None]
    r_idx = np.arange(128)[None, :]
    masks = np.zeros((128, 256), dtype=np.float32)
    masks[:, 0:128] = (r_idx < s_idx)    # M2: window edge block
    masks[:, 128:256] = (r_idx >= s_idx)  # M1: causal diagonal block
    masks = masks.astype(bf16)

    in_maps = []
    for c in range(N_CORES):
        lo, hi = c * PAIRS_PER_CORE, (c + 1) * PAIRS_PER_CORE
        in_maps.append({
            "qT": qT[lo:hi], "kT": kT[lo:hi], "vext": vext[lo:hi],
            "masks": masks,
        })

    nc = _get_nc()
    trace = _TRACE and _ensure_ntff_hook()
    res = run_bass_kernel_spmd(
        nc, in_maps, core_ids=list(range(N_CORES)), trace=trace)
    LAST_RUN_INFO["exec_time_ns"] = res.exec_time_ns
    LAST_RUN_INFO["mean_exec_time_ns"] = res.mean_exec_time_ns
    LAST_RUN_INFO["profile_json"] = res.profile_json

    # Gather + normalize + undo layouts on host.
    raw = np.concatenate(
        [np.asarray(res.results[c]["out"]) for c in range(N_CORES)], axis=0
    ).astype(np.float32)                              # [pairs, 128, NQB*129]
    raw = raw.reshape(npairs, 128, NQB, VSLOT)
    num = raw[:, :, :, :D]                            # [pairs, r, qi, d]
    den = raw[:, :, :, D:D + 1]
    out = (num / den).transpose(0, 2, 1, 3)           # [pairs, qi, r, d]
    return np.ascontiguousarray(
        out.reshape(B, H, T, D).astype(np.float32))


# revision 10
# speedup vs baseline: 1.0767x; 1.0016x over previous
"""Sliding-window causal attention (window=512) on 8 TRN2 NeuronCores.

Full inputs q,k,v: [4, 16, 2048, 128] fp32. B*H = 64 (batch, head) pairs are
sharded 8-per-core (head/batch parallel, no cross-core communication).

Per (pair, 128-query-block) on device:
  - <=5 QK^T matmuls (bf16) produce transposed scores S^T[key, q] in PSUM
    (key on partitions so the AV matmul needs no on-chip transpose).
  - one Exp over the whole score block (scores are O(1): q is pre-scaled by
    1/sqrt(d) on host, so no max-subtraction is needed).
  - triangular 0/1 mask multiplies on the first (window-edge) and diagonal
    (causal) key blocks.
  - <=5 accumulating AV matmuls: out[q, 0:128] = P^T.T @ v, out[q, 128] = sum
    of probs (denominator) via a ones-column appended to v on host.
  - normalization (divide by out[:, 128]) happens on host after gather.

Host-side prep/post (numpy) handles the [T,d] -> [d,T] transposes, bf16
casts, and the final division - none of which touch the device.
"""

import os

import ml_dtypes
import numpy as np

from concourse import bacc, bass, mybir, tile
from concourse.bass_utils import run_bass_kernel_spmd

B, H, T, D = 4, 16, 2048, 128
WINDOW = 512
SCALE = D ** -0.5
N_CORES = 8
PAIRS_PER_CORE = (B * H) // N_CORES  # 8
NQB = T // 128                       # 16 query blocks of 128 per pair
NKB = T // 128                       # 16 key blocks of 128 per pair
VSLOT = 129                          # v block width + ones column
BF16 = mybir.dt.bfloat16
F32 = mybir.dt.float32

_TRACE = bool(int(os.environ.get("KERNEL_TRACE", "0")))
LAST_RUN_INFO = {}


def _ensure_ntff_hook():
    """The agent image's ``antenv`` lacks ``axon_hooks``, so concourse's
    trace path can't find the NTFF profile hook. Synthesize the module and
    register the ctypes-based hook from trn_agent_boot."""
    import sys
    import types

    try:
        from antenv.axon_hooks import get_axon_ntff_profile_hook  # noqa: F401
        return True
    except ImportError:
        pass
    try:
        import antenv
        from trn_agent_boot.trn_boot import _ntff_profile_via_ctypes

        hook = _ntff_profile_via_ctypes("/opt/axon/libaxon_pjrt.so")
        mod = types.ModuleType("antenv.axon_hooks")
        _state = {"hook": hook}
        mod.set_axon_ntff_profile_hook = lambda h: _state.__setitem__("hook", h)
        mod.get_axon_ntff_profile_hook = lambda: _state["hook"]
        sys.modules["antenv.axon_hooks"] = mod
        antenv.axon_hooks = mod
        return hook is not None
    except Exception:
        return False


def _build_bass():
    # bacc.Bacc (not bass.Bass): its finalize() runs
    # generate_event_semaphores(), which splits multi-sem waits to satisfy
    # the TRN2 one-wait-per-instruction constraint walrus enforces.
    nc = bacc.Bacc()
    qT_ext = nc.declare_dram_parameter(
        "qT", [PAIRS_PER_CORE, 128, T], BF16, isOutput=False)
    kT_ext = nc.declare_dram_parameter(
        "kT", [PAIRS_PER_CORE, 128, T], BF16, isOutput=False)
    v_ext = nc.declare_dram_parameter(
        "vext", [PAIRS_PER_CORE, 128, NKB * VSLOT], BF16, isOutput=False)
    m_ext = nc.declare_dram_parameter("masks", [128, 256], BF16, isOutput=False)
    out_ext = nc.declare_dram_parameter(
        "out", [PAIRS_PER_CORE, 128, NQB * VSLOT], BF16, isOutput=True)

    with tile.TileContext(nc) as tc:
        with (
            tc.tile_pool(name="qk_in", bufs=2) as qk_pool,
            tc.tile_pool(name="v_in", bufs=2) as v_pool,
            tc.tile_pool(name="mask", bufs=1) as mask_pool,
            tc.tile_pool(name="probs", bufs=3) as probs_pool,
            tc.tile_pool(name="stage", bufs=2) as stage_pool,
            tc.tile_pool(name="scores", bufs=2, space="PSUM") as scores_pool,
            tc.tile_pool(name="outp", bufs=2, space="PSUM") as outp_pool,
        ):
            mask_t = mask_pool.tile([128, 256], BF16)
            nc.sync.dma_start(mask_t[:], m_ext[:])
            m2 = mask_t[:, 0:128]    # window edge block: valid iff r < s
            m1 = mask_t[:, 128:256]  # diagonal block: valid iff r >= s

            for p in range(PAIRS_PER_CORE):
                qt = qk_pool.tile([128, T], BF16, tag="qt")
                nc.sync.dma_start(qt[:], qT_ext[p])
                kt = qk_pool.tile([128, T], BF16, tag="kt")
                nc.sync.dma_start(kt[:], kT_ext[p])
                vt = v_pool.tile([128, NKB * VSLOT], BF16, tag="vt")
                nc.sync.dma_start(vt[:], v_ext[p])
                stage = stage_pool.tile([128, NQB * VSLOT], BF16, tag="stage")

                # Two query blocks per iteration: one exp + one PSUM out tile
                # + one stage copy per super-block halves ACT/DVE op-count
                # overhead and semaphore traffic.
                for qs in range(NQB // 2):
                    qiA, qiB = 2 * qs, 2 * qs + 1
                    kb0A = max(0, qiA - 4)
                    kb0B = max(0, qiB - 4)
                    # Score layout (key on partitions, q on free dim):
                    #   [shared kbs kb0B..qiA, each 256 = A-half | B-half]
                    #   [B-only diag qiB (128)]
                    #   [A-only edge kb0A (128, absent when kb0A==kb0B)]
                    # Shared kbs use ONE N=256 matmul covering both q blocks;
                    # 256-wide slices sit at 256-aligned cols so no matmul
                    # crosses a PSUM bank boundary.
                    a_only = kb0B - kb0A            # 0 or 1
                    nsh = qiA - kb0B + 1
                    wtot = nsh * 256 + 128 + a_only * 128

                    def acol(kb):  # column of the A-half for key block kb
                        if a_only and kb == kb0A:
                            return nsh * 256 + 128
                        return (kb - kb0B) * 256

                    def bcol(kb):  # column of the B-half for key block kb
                        if kb == qiB:
                            return nsh * 256
                        return (kb - kb0B) * 256 + 128

                    scores = scores_pool.tile([128, wtot], F32, tag="scores")
                    for j in range(nsh):
                        nc.tensor.matmul(
                            scores[:, j * 256:(j + 1) * 256],
                            lhsT=kt[:, (kb0B + j) * 128:(kb0B + j + 1) * 128],
                            rhs=qt[:, qiA * 128:(qiA + 2) * 128],
                            start=True, stop=True,
                        )
                    nc.tensor.matmul(
                        scores[:, nsh * 256:nsh * 256 + 128],
                        lhsT=kt[:, qiB * 128:(qiB + 1) * 128],
                        rhs=qt[:, qiB * 128:(qiB + 1) * 128],
                        start=True, stop=True,
                    )
                    if a_only:
                        nc.tensor.matmul(
                            scores[:, nsh * 256 + 128:wtot],
                            lhsT=kt[:, kb0A * 128:(kb0A + 1) * 128],
                            rhs=qt[:, qiA * 128:(qiA + 1) * 128],
                            start=True, stop=True,
                        )

                    probs = probs_pool.tile([128, wtot], BF16, tag="probs")
                    nc.scalar.activation(
                        probs[:], scores[:], mybir.ActivationFunctionType.Exp)

                    def two_block_view(ap_full, col0, step):
                        base = ap_full[:, col0:col0 + 128]
                        return bass.AP(
                            base.tensor, base.offset,
                            [base.ap[0], [step, 2], [1, 128]])

                    dA, dB = acol(qiA), bcol(qiB)
                    m1b = bass.AP(
                        m1.tensor, m1.offset, [m1.ap[0], [0, 2]] + list(m1.ap[1:]))
                    diag2 = two_block_view(probs, dA, dB - dA)
                    nc.vector.tensor_mul(diag2, diag2, m1b)
                    if qiA >= 4:
                        eA, eB = acol(kb0A), bcol(kb0B)
                        lo, hi = min(eA, eB), max(eA, eB)
                        m2b = bass.AP(
                            m2.tensor, m2.offset, [m2.ap[0], [0, 2]] + list(m2.ap[1:]))
                        edge2 = two_block_view(probs, lo, hi - lo)
                        nc.vector.tensor_mul(edge2, edge2, m2b)

                    outp = outp_pool.tile([128, 2 * VSLOT], F32, tag="outp")
                    for i, kb in enumerate(range(kb0A, qiA + 1)):
                        c = acol(kb)
                        nc.tensor.matmul(
                            outp[:, 0:VSLOT],
                            lhsT=probs[:, c:c + 128],
                            rhs=vt[:, kb * VSLOT:(kb + 1) * VSLOT],
                            start=(i == 0), stop=(kb == qiA),
                        )
                    for i, kb in enumerate(range(kb0B, qiB + 1)):
                        c = bcol(kb)
                        nc.tensor.matmul(
                            outp[:, VSLOT:2 * VSLOT],
                            lhsT=probs[:, c:c + 128],
                            rhs=vt[:, kb * VSLOT:(kb + 1) * VSLOT],
                            start=(i == 0), stop=(kb == qiB),
                        )
                    nc.vector.tensor_copy(
                        stage[:, qiA * VSLOT:(qiA + 2) * VSLOT], outp[:])

                nc.sync.dma_start(out_ext[p], stage[:])

    # Run bacc's lowering (register allocation + sem-wait legalization);
    # run_bass_via_pjrt serializes without finalizing.
    nc.finalize()
    return nc


_NC_CACHE = None


def _get_nc():
    global _NC_CACHE
    if _NC_CACHE is None:
        _NC_CACHE = _build_bass()
    return _NC_CACHE


def kernel(q, k, v):
    q = np.asarray(q, dtype=np.float32)
    k = np.asarray(k, dtype=np.float32)
    v = np.asarray(v, dtype=np.float32)
    bf16 = ml_dtypes.bfloat16

    npairs = B * H
    # [pairs, d, T] transposed layouts for the QK^T matmul; q pre-scaled.
    qT = np.ascontiguousarray(
        (q.reshape(npairs, T, D) * SCALE).transpose(0, 2, 1)).astype(bf16)
    kT = np.ascontiguousarray(
        k.reshape(npairs, T, D).transpose(0, 2, 1)).astype(bf16)
    # v blocks in natural layout + ones column: vext[p, s, kb*129 + c]
    vext = np.ones((npairs, 128, NKB, VSLOT), dtype=np.float32)
    vext[:, :, :, :D] = v.reshape(npairs, NKB, 128, D).transpose(0, 2, 1, 3)
    vext = vext.reshape(npairs, 128, NKB * VSLOT).astype(bf16)

    s_idx = np.arange(128)[:, None]
    r_idx = np.arange(128)[None, :]
    masks = np.zeros((128, 256), dtype=np.float32)
    masks[:, 0:128] = (r_idx < s_idx)    # M2: window edge block
    masks[:, 128:256] = (r_idx >= s_idx)  # M1: causal diagonal block
    masks = masks.astype(bf16)

    in_maps = []
    for c in range(N_CORES):
        lo, hi = c * PAIRS_PER_CORE, (c + 1) * PAIRS_PER_CORE
        in_maps.append({
            "qT": qT[lo:hi], "kT": kT[lo:hi], "vext": vext[lo:hi],
            "masks": masks,
        })

    nc = _get_nc()
    trace = _TRACE and _ensure_ntff_hook()
    res = run_bass_kernel_spmd(
        nc, in_maps, core_ids=list(range(N_CORES)), trace=trace)
    LAST_RUN_INFO["exec_time_ns"] = res.exec_time_ns
    LAST_RUN_INFO["mean_exec_time_ns"] = res.mean_exec_time_ns
    LAST_RUN_INFO["profile_json"] = res.profile_json

    # Gather + normalize + undo layouts on host.
    raw = np.concatenate(
        [np.asarray(res.results[c]["out"]) for c in range(N_CORES)], axis=0
    ).astype(np.float32)                              # [pairs, 128, NQB*129]
    raw = raw.reshape(npairs, 128, NQB, VSLOT)
    num = raw[:, :, :, :D]                            # [pairs, r, qi, d]
    den = raw[:, :, :, D:D + 1]
    out = (num / den).transpose(0, 2, 1, 3)           # [pairs, qi, r, d]
    return np.ascontiguousarray(
        out.reshape(B, H, T, D).astype(np.float32))
